# revision 1
# baseline (speedup 1.0000x reference)
"""Bass/Trainium2 kernel for nn_ConcatenationFusionLayer_29575144801128.

Math: out = inputs.reshape(65536, 1024) where inputs is a contiguous
(65536, 2, 512) f32 tensor -- i.e. the output bytes are identical to the
input bytes.  The kernel is therefore a pure HBM->HBM memcpy, done
data-parallel across 8 NeuronCores (batch dim sharded, 8192 rows = 32 MiB
per core).  Each core issues chunked DRAM->DRAM DMA copies (no SBUF
round-trip needed), split across the two HWDGE rings (sync + scalar).
"""

import numpy as np

N_CORES = 8
B = 65536
FLAT = 1024  # 2 * 512
PER_CORE = B // N_CORES  # 8192 rows -> 32 MiB per core

# Number of dma_start chunks per core; even chunks go on nc.sync's HWDGE
# ring, odd chunks on nc.scalar's.  Two rings keep all 16 SDMA engines fed
# across per-DMA completion stalls (engines round-robin between rings at
# packet granularity); 16 chunks x 2 MiB measured best cold-start.
N_CHUNKS = 16

_cache = {}


def _build_nc():
    import concourse.bass as bass
    import concourse.mybir as mybir

    nc = bass.Bass()
    x = nc.declare_dram_parameter(
        "x", [PER_CORE, FLAT], mybir.dt.float32, isOutput=False
    )
    y = nc.declare_dram_parameter(
        "y", [PER_CORE, FLAT], mybir.dt.float32, isOutput=True
    )

    total = PER_CORE * FLAT  # elements per core
    assert total % N_CHUNKS == 0
    chunk = total // N_CHUNKS

    with (
        nc.Block() as block,
        nc.semaphore("dma_sem") as dma_sem,
    ):

        @block.sync
        def _(sync):
            for i in range(0, N_CHUNKS, 2):
                sync.dma_start(
                    out=bass.AP(y, i * chunk, [[1, chunk]]),
                    in_=bass.AP(x, i * chunk, [[1, chunk]]),
                ).then_inc(dma_sem, 16)
            # wait for ALL chunks (both engines' DMAs) to land
            sync.wait_ge(dma_sem, 16 * N_CHUNKS)

        @block.scalar
        def _(scalar):
            for i in range(1, N_CHUNKS, 2):
                scalar.dma_start(
                    out=bass.AP(y, i * chunk, [[1, chunk]]),
                    in_=bass.AP(x, i * chunk, [[1, chunk]]),
                ).then_inc(dma_sem, 16)

    return nc


def _run(inputs_arr: np.ndarray, **spmd_kwargs):
    """Shard, run on 8 cores, gather.  Returns (out, BassKernelResults)."""
    from concourse.bass_utils import run_bass_kernel_spmd

    x = np.ascontiguousarray(np.asarray(inputs_arr, dtype=np.float32))
    assert x.shape == (B, 2, 512), x.shape
    shards = x.reshape(N_CORES, PER_CORE, FLAT)

    if "nc" not in _cache:
        _cache["nc"] = _build_nc()
    nc = _cache["nc"]

    in_maps = [{"x": shards[i]} for i in range(N_CORES)]
    res = run_bass_kernel_spmd(nc, in_maps, core_ids=list(range(N_CORES)), **spmd_kwargs)
    out = np.concatenate([r["y"] for r in res.results], axis=0)
    return out, res


def kernel(**inputs) -> np.ndarray:
    out, _ = _run(inputs["inputs"])
    return out



# revision 2
# speedup vs baseline: 1.1407x; 1.1407x over previous
"""Bass/Trainium2 kernel for nn_ConcatenationFusionLayer_29575144801128.

Math: out = inputs.reshape(65536, 1024) where inputs is a contiguous
(65536, 2, 512) f32 tensor -- the output bytes are identical to the input
bytes, so the kernel is a pure HBM->HBM memcpy, data-parallel across 8
NeuronCores (batch dim sharded, 8192 rows = 32 MiB per core).  Each core
issues DRAM->DRAM DMA copies (no SBUF round-trip) on the two HWDGE rings
(sync + scalar).

Per-core layout.  The copy is bound by the 16 SDMA engines' DRAM->DRAM
streaming rate (~21 GB/s payload per engine, ~340 GB/s per core, all 16
engines ~100% busy), so the only free variable is how the 512 64-KiB rows
are spread over engines.  Profiled runs frequently show one engine (local
engine 15) losing ~20% of its bandwidth to periodic runtime/profiling DMA
traffic, making it a straggler that adds ~20 us to the span.  The HWDGE
descriptor generator splits any CONTIGUOUS transfer evenly across all 16
engines, but maps the rows of a non-mergeable strided AP 1:1 onto engines
0..nrows-1 (row j -> engine j).  We exploit that to give engine 15 ~31%
less work, so it finishes with the pack even when contended:

  rows   0..351  3 contiguous chunks (128+128+96 rows) -> even 1/16 spread
  rows 352..501  10 strided "columns" (stride 10 rows, 64-KiB rows,
                 nrows=15): column c covers rows 352+c+10j, j=0..14;
                 row j -> engine j, so engine 15 gets nothing
  rows 502..511  (the would-be j=15 rows) 1 contiguous chunk -> even

Per-engine load: engines 0-14 get 32.625 rows, engine 15 gets 22.625
(69%, i.e. it tolerates ~31% bandwidth loss before becoming the
straggler).  Clean-run cost of the insurance is ~0.6% extra on engines
0-14.  Measured: ~113.5 us stable vs a balanced layout's bimodal
~113 / ~133 us distribution.
"""

import numpy as np

N_CORES = 8
B = 65536
FLAT = 1024  # 2 * 512
PER_CORE = B // N_CORES  # 8192 rows -> 32 MiB per core

ROW = 16384   # 64-KiB DMA row, in f32 elements (max SDMA descriptor)
N_COLS = 10   # strided columns; engine-15 deficit = N_COLS rows

_cache = {}


def _instruction_lists():
    """Two per-ring lists of (offset_elems, access_pattern).
    Together they cover rows 0..511 exactly once."""
    # bulk: rows 0..351, even engine spread
    bulk = [
        (0 * ROW, [[1, 128 * ROW]]),
        (128 * ROW, [[1, 128 * ROW]]),
        (256 * ROW, [[1, 96 * ROW]]),
    ]
    # strided columns: rows 352..501, engines 0..14 only
    cols = [
        ((352 + c) * ROW, [[10 * ROW, 15], [1, ROW]]) for c in range(N_COLS)
    ]
    # tail: rows 502..511, even engine spread
    tail = [(502 * ROW, [[1, N_COLS * ROW]])]

    ring_a = [bulk[0], bulk[2]] + cols[5:]
    ring_b = [bulk[1]] + cols[:5] + tail
    return ring_a, ring_b


def _build_nc():
    import concourse.bass as bass
    import concourse.mybir as mybir

    nc = bass.Bass()
    x = nc.declare_dram_parameter(
        "x", [PER_CORE, FLAT], mybir.dt.float32, isOutput=False
    )
    y = nc.declare_dram_parameter(
        "y", [PER_CORE, FLAT], mybir.dt.float32, isOutput=True
    )

    ring_a, ring_b = _instruction_lists()
    n_instr = len(ring_a) + len(ring_b)

    with (
        nc.Block() as block,
        nc.semaphore("dma_sem") as dma_sem,
    ):
        def make(instrs, is_sync):
            def body(eng):
                for off, ap in instrs:
                    eng.dma_start(
                        out=bass.AP(y, off, [list(d) for d in ap]),
                        in_=bass.AP(x, off, [list(d) for d in ap]),
                    ).then_inc(dma_sem, 16)
                if is_sync:
                    eng.wait_ge(dma_sem, 16 * n_instr)
            return body

        block.sync(make(ring_a, True))
        block.scalar(make(ring_b, False))
    return nc


def _run(inputs_arr: np.ndarray, **spmd_kwargs):
    """Shard, run on 8 cores, gather.  Returns (out, BassKernelResults)."""
    from concourse.bass_utils import run_bass_kernel_spmd

    x = np.ascontiguousarray(np.asarray(inputs_arr, dtype=np.float32))
    assert x.shape == (B, 2, 512), x.shape
    shards = x.reshape(N_CORES, PER_CORE, FLAT)

    if "nc" not in _cache:
        _cache["nc"] = _build_nc()
    nc = _cache["nc"]

    in_maps = [{"x": shards[i]} for i in range(N_CORES)]
    res = run_bass_kernel_spmd(nc, in_maps, core_ids=list(range(N_CORES)), **spmd_kwargs)
    out = np.concatenate([r["y"] for r in res.results], axis=0)
    return out, res


def kernel(**inputs) -> np.ndarray:
    out, _ = _run(inputs["inputs"])
    return out
